# revision 32
# baseline (speedup 1.0000x reference)
"""Trainium2 Bass kernel for nn_Attention_42417097015520.

Full-input contract: kernel(**inputs) takes the unsharded inputs
(x [4,2048,768], W_qkv [768,2304], W_proj [768,768], b_proj [768]) and
returns the full [4,2048,768] output.

Sharding (8 cores): core c handles batch b=c//2 and heads
h in [(c%2)*6, (c%2)*6+6) (tensor parallel over heads x data parallel
over batch). Each core computes its 6 heads' attention plus the partial
output projection against its 384-row slice of W_proj; the host sums the
two partials per batch and adds b_proj.

Device-side layout/algorithm (per core, identical SPMD program, bf16
matmul operands, fp32 PSUM accumulation):
  - inputs: xT = x[b].T [768,2048], wqkv = W_qkv column slice [768,1152]
    (q|k|v blocks of 384), wproj row slice [384,768].
  - Q^T/K^T projections in head-pair-packed planes (head h on partitions
    (h%2)*64.. of plane h//2); V with a ones column per head (so the P@V
    matmul also emits the softmax denominators as PSUM row 64) and junk
    padding so its lhsT can be taken 128 wide (full PE array keeps the
    HAM clock-gate warm).
  - Attention blocks (n-block x head-plane): S^T chunks for both heads of
    a plane issued as row-tiled matmuls at partition bases 0/64 (disjoint
    row groups run concurrently), exp on ScalarE (scale=1/8 folded in,
    two chunks per activation), O^T accumulated over m-chunks, then
    normalized by the reciprocal denominator row broadcast via a DRAM
    bounce. All remaining production (K/V/Q projections, output
    projection) is drained as "extra PE work" between attention groups
    from a JIT queue, and each block's first S-matmuls are emitted ahead
    of the previous block's tail, so ScalarE (the ~200us exp floor) and
    the PE stay busy simultaneously.
  - Output projection from the O^T layout (heads on partitions), partial
    result [2048,768] DMA'd out; host sums batch partials + b_proj.
"""

import sys
import types
import contextlib
import ctypes
from contextlib import ExitStack

import numpy as np

import concourse.bass as bass
import concourse.mybir as mybir
import concourse.tile as tile
from concourse.bass_utils import run_bass_kernel_spmd

B, N, D, H, HD = 4, 2048, 768, 12, 64
HPC = H // 2          # heads per core = 6
NCORES = 8
SCALE = HD ** -0.5    # 0.125
F32 = mybir.dt.float32
F32R = mybir.dt.float32r
BF16 = mybir.dt.bfloat16
P = 128
VW = P                # V columns per head: 64 data + 64 ones (denominator rows)


# ---------------------------------------------------------------------------
# Workaround: this container's walrus accepts at most ONE sem wait per
# instruction. Hoist extra waits onto same-engine NoOps inserted before.
# ---------------------------------------------------------------------------
_wsplit_ctr = [0]


def _split_waits(nc, cap: int = 1) -> int:
    n_split = 0
    for f in nc.m.functions:
        for bb in f.blocks:
            insts = list(bb.instructions)
            out = []
            for ins in insts:
                si = ins.sync_info
                if si is not None and si.on_wait and len(si.on_wait) > cap:
                    waits = list(si.on_wait)
                    for i in range(0, len(waits) - cap, cap):
                        _wsplit_ctr[0] += 1
                        out.append(
                            mybir.InstNoOp(
                                name=f"I-wsplit-{_wsplit_ctr[0]}",
                                engine=ins.engine,
                                ins=[],
                                outs=[],
                                sync_info=mybir.SyncInfo(
                                    on_wait=waits[i : i + cap], on_update=[]
                                ),
                            )
                        )
                    si.on_wait = waits[len(waits) - cap :]
                    n_split += 1
                out.append(ins)
            if len(out) != len(insts):
                bb.instructions[:] = out
    return n_split


# ---------------------------------------------------------------------------
# Post-pass: every matmul/activation/copy carries a +1 sem update, but only
# the counts some wait actually references need an update (each serialized
# inc costs ~26 ns on the engine).  Keep updates only at wait thresholds,
# bumping their increment to cover the dropped ones, so every wait fires at
# exactly the same instruction completion as before.
# ---------------------------------------------------------------------------
def _coalesce_sem_updates(nc, sems=("PE_44", "Activation_44", "DVE_44")):
    thr = {s: set() for s in sems}
    ok = {s: True for s in sems}
    streams = {s: [] for s in sems}
    waiters = {s: [] for s in sems}
    engines = {s: set() for s in sems}
    for f in nc.m.functions:
        for bb in f.blocks:
            for ins in bb.instructions:
                si = ins.sync_info
                if not si:
                    continue
                for w in (si.on_wait or []):
                    if w.ant_name in thr:
                        if w.wait_mode != "sem-ge-imm" or w.wait_reg is not None:
                            ok[w.ant_name] = False
                        else:
                            thr[w.ant_name].add(w.wait_value)
                            waiters[w.ant_name].append(w)
                for u in (si.on_update or []):
                    if u.ant_name in streams:
                        if u.update_mode != "sem-inc" or u.update_value != 1:
                            ok[u.ant_name] = False
                        streams[u.ant_name].append((ins, u))
                        engines[u.ant_name].add(str(ins.engine))
    removed = 0
    # Keep a +1 update only at counts some wait references, and renumber all
    # wait values to the rank of their threshold.  Updates stay attached to
    # their original instruction (so they still fire at its completion) and
    # stay sem-inc (no add-imm, which shares the ISA immediate with ge-imm
    # waits); every wait fires at exactly the same completion as before.
    for s in sems:
        lst = streams[s]
        if not ok[s] or len(engines[s]) != 1 or not lst:
            continue
        total = len(lst)
        T = sorted(v for v in thr[s] if 0 < v <= total)
        rank = {v: i + 1 for i, v in enumerate(T)}
        keep = set(T)
        for c, (ins, u) in enumerate(lst, start=1):
            if c not in keep:
                ins.sync_info.on_update = [
                    x for x in ins.sync_info.on_update if x is not u
                ]
                removed += 1
        seen = set()
        for w in waiters[s]:
            if id(w) in seen:
                continue
            seen.add(id(w))
            if 0 < w.wait_value <= total:
                w.wait_value = rank[w.wait_value]
    return removed


# ---------------------------------------------------------------------------
# Post-pass: each S-pair emits two half-height (64,128) LDWEIGHTS for the
# row-tiled matmuls; a half-height load is not FWL-eligible and cannot hide
# behind a full-height matmul (row conflict), exposing ~107ns per pair.
# Merge each pair into ONE full-height (128,128) LDWEIGHTS covering both
# tiles' rows (the same contiguous SBUF region), which hides like the O
# matmuls' loads do.
# ---------------------------------------------------------------------------
def _merge_s_ldw(nc):
    n = 0
    for f in nc.m.functions:
        for bb in f.blocks:
            insts = bb.instructions
            out = []
            i = 0
            while i < len(insts):
                ins = insts[i]
                if (
                    type(ins).__name__ == "InstLdweights"
                    and getattr(ins, "tile_size", None) == (64, 128)
                    and ins.tile_position == (0, 0)
                ):
                    stride = ins.ins[0].ap[0][0]
                    for j in range(i + 1, min(i + 5, len(insts))):
                        p = insts[j]
                        if (
                            type(p).__name__ == "InstLdweights"
                            and getattr(p, "tile_size", None) == (64, 128)
                            and p.tile_position == (64, 0)
                            and p.ins[0].offset == ins.ins[0].offset + 64 * stride
                            and (p.sync_info is None
                                 or (not p.sync_info.on_wait
                                     and not p.sync_info.on_update))
                        ):
                            break
                    else:
                        out.append(ins)
                        i += 1
                        continue
                    ins.ins[0].ap = [[stride, 128], [1, 128]]
                    ins.tile_size = (128, 128)
                    out.append(ins)
                    for k in range(i + 1, j):
                        out.append(insts[k])
                    i = j + 1
                    n += 1
                    continue
                out.append(ins)
                i += 1
            bb.instructions[:] = out
    return n


# ---------------------------------------------------------------------------
# NTFF profiling shim (the image's antenv lacks axon_hooks); only needed
# when trace=True is requested.
# ---------------------------------------------------------------------------
_HOOK = [None]


def _install_ntff_shim():
    if "antenv.axon_hooks" in sys.modules:
        return
    mod = types.ModuleType("antenv.axon_hooks")
    mod.set_axon_ntff_profile_hook = lambda h: _HOOK.__setitem__(0, h)
    mod.get_axon_ntff_profile_hook = lambda: _HOOK[0]
    sys.modules["antenv.axon_hooks"] = mod
    try:
        import antenv

        antenv.axon_hooks = mod
    except ImportError:
        pass

    try:
        lib = ctypes.CDLL("/opt/axon/libaxon_pjrt.so")
    except OSError:
        return
    if not hasattr(lib, "axon_start_nrt_profile"):
        return
    lib.axon_start_nrt_profile.argtypes = [
        ctypes.POINTER(ctypes.c_int64),
        ctypes.c_size_t,
    ]
    lib.axon_start_nrt_profile.restype = ctypes.c_int64
    lib.axon_stop_nrt_profile.argtypes = [ctypes.c_char_p]
    lib.axon_stop_nrt_profile.restype = ctypes.c_int64

    @contextlib.contextmanager
    def _hook(output_dir, device_ids):
        import jax

        jax.devices()
        if device_ids:
            ids = (ctypes.c_int64 * len(device_ids))(*device_ids)
            rc = lib.axon_start_nrt_profile(ids, len(device_ids))
        else:
            rc = lib.axon_start_nrt_profile(None, 0)
        if rc != 0:
            raise RuntimeError(f"axon_start_nrt_profile rc={rc}")
        try:
            yield
        finally:
            n = lib.axon_stop_nrt_profile(str(output_dir).encode())
            if n < 0:
                raise RuntimeError(f"axon_stop_nrt_profile rc={n}")

    _HOOK[0] = _hook

    import concourse.bass_utils as bu

    bu.upload_artifacts = lambda tmpdir: str(tmpdir)


# ---------------------------------------------------------------------------
# Device program
# ---------------------------------------------------------------------------
def _build_nc():
    nc = bass.Bass()
    xT = nc.declare_dram_parameter("xT", [D, N], BF16, isOutput=False).ap()
    wqkv = nc.declare_dram_parameter("wqkv", [D, 3 * HPC * HD], BF16, isOutput=False).ap()
    wproj = nc.declare_dram_parameter("wproj", [HPC * HD, D], BF16, isOutput=False).ap()
    out = nc.declare_dram_parameter("out", [N, D], F32, isOutput=True).ap()

    DO = D // P          # 6 d-chunks of 128
    NB = N // 512        # 4 n-blocks of 512
    MC = N // P          # 16 m-chunks of 128
    PH = HPC * HD // P   # 3 planes of head-dims

    with tile.TileContext(nc) as tc, ExitStack() as ctx:
        persist = ctx.enter_context(tc.tile_pool(name="persist", bufs=1))
        ptp = ctx.enter_context(tc.tile_pool(name="ptp", bufs=2))
        outcp = ctx.enter_context(tc.tile_pool(name="outcp", bufs=3))
        small = ctx.enter_context(tc.tile_pool(name="small", bufs=8))
        dramp = ctx.enter_context(tc.tile_pool(name="dramp", bufs=8, space="DRAM"))
        psum_mm = ctx.enter_context(tc.tile_pool(name="psum_mm", bufs=2, space="PSUM"))
        psum_s = ctx.enter_context(tc.tile_pool(name="psum_s", bufs=1, space="PSUM"))
        psum_o = ctx.enter_context(tc.tile_pool(name="psum_o", bufs=2, space="PSUM"))

        # Q^T and K^T planes use head-pair packing: head h lives on
        # partitions (h%2)*64.. of plane h//2. The S^T matmuls for the two
        # heads of a plane are emitted back-to-back as row-tiled (base
        # partition 0 / 64) matmuls, so they run CONCURRENTLY on disjoint
        # row groups of the PE array -- 2x throughput, and the combined
        # activity keeps the HAM clock-gate at full speed.
        qT_sb = persist.tile([P, PH, N], BF16)                   # [128, 3, 2048]
        kT_sb = persist.tile([P, PH, N], BF16)                   # [128, 3, 2048]
        # V tile: per head 64 V columns + 64 ONES columns, so each O^T matmul
        # (lhsT 128 wide) emits the softmax denominator replicated on PSUM
        # rows 64..127 for free -- normalization then needs no cross-partition
        # broadcast, just a DVE reciprocal + multiply straight from PSUM.
        v_sb = persist.tile([P, MC, HPC * P], BF16)              # [128, 16, 768]
        oT_sb = persist.tile([P, PH, N], BF16)                   # [128, 3, 2048]
        wp_sb = persist.tile([P, PH, D], BF16)                   # [128, 3, 768]
        xT_sb = persist.tile([P, DO, N], BF16)                   # [128, 6, 2048]
        wqkv_sb = persist.tile([P, DO, 3 * HPC * HD], BF16)      # [128, 6, 1152]

        # hoist the ACT table load (~2.7us) into the DMA prologue; its memset
        # must be DVE's first op so the dummy activation issues immediately
        warm = small.tile([1, 8], F32)
        nc.vector.memset(warm[:, :], 0.0)
        nc.scalar.activation(warm[:, :], warm[:, :],
                             mybir.ActivationFunctionType.Exp, scale=1.0)

        for h in range(HPC):
            nc.vector.memset(v_sb[:, :, h * P + HD:(h + 1) * P], 1.0)

        # DMA order: first the K-plane-0 wqkv slice (cols 384:512, the first
        # matmul's weights) and the first x quarter, then Q cols, then the
        # remaining K planes, V, and the rest of x.
        QK = 2 * HPC * HD
        for o in range(DO):
            nc.gpsimd.dma_start(out=wqkv_sb[:, o, PH * P:(PH + 1) * P],
                                in_=wqkv[o * P:(o + 1) * P, PH * P:(PH + 1) * P])
            nc.sync.dma_start(out=xT_sb[:, o, 0:512], in_=xT[o * P:(o + 1) * P, 0:512])
        for o in range(DO):
            nc.gpsimd.dma_start(out=wqkv_sb[:, o, 0:PH * P],
                                in_=wqkv[o * P:(o + 1) * P, 0:PH * P])
        for o in range(DO):
            nc.gpsimd.dma_start(out=wqkv_sb[:, o, (PH + 1) * P:QK],
                                in_=wqkv[o * P:(o + 1) * P, (PH + 1) * P:QK])
        for o in range(DO):
            nc.gpsimd.dma_start(out=wqkv_sb[:, o, QK:], in_=wqkv[o * P:(o + 1) * P, QK:])
        for o in range(DO):
            nc.sync.dma_start(out=xT_sb[:, o, 512:N // 2], in_=xT[o * P:(o + 1) * P, 512:N // 2])
        for o in range(DO):
            nc.sync.dma_start(out=xT_sb[:, o, N // 2:N], in_=xT[o * P:(o + 1) * P, N // 2:N])
        for p3 in range(PH):
            nc.sync.dma_start(out=wp_sb[:, p3, :], in_=wproj[p3 * P:(p3 + 1) * P, :])

        def qk_proj(cb, nb):
            """Produce one [128,512] column-block of Q^T (cb<3) or K^T."""
            ps = psum_mm.tile([P, 512], F32, tag="mmps")
            for o in range(DO):
                nc.tensor.matmul(
                    ps[:, :],
                    lhsT=wqkv_sb[:, o, cb * P:(cb + 1) * P],
                    rhs=xT_sb[:, o, nb * 512:(nb + 1) * 512],
                    start=(o == 0),
                    stop=(o == DO - 1),
                )
            sl = slice(nb * 512, (nb + 1) * 512)
            if cb < PH:
                nc.vector.tensor_copy(qT_sb[:, cb, sl], ps[:, :])
            else:
                nc.vector.tensor_copy(kT_sb[:, cb - PH, sl], ps[:, :])

        def v_proj(mc):
            ps = psum_mm.tile([P, 512], F32, tag="mmps")
            for o in range(DO):
                nc.tensor.matmul(
                    ps[:, : HPC * HD],
                    lhsT=xT_sb[:, o, mc * P:(mc + 1) * P],
                    rhs=wqkv_sb[:, o, 2 * HPC * HD: 3 * HPC * HD],
                    start=(o == 0),
                    stop=(o == DO - 1),
                )
            nc.vector.tensor_copy(
                v_sb.rearrange("p m (h c) -> p m h c", c=P)[:, mc, :, 0:HD],
                ps[:, : HPC * HD].rearrange("p (h c) -> p h c", c=HD),
            )

        def proj(nb):
            """Output projection for one 512-row n-block."""
            for mcl in range(512 // P):
                mc = nb * (512 // P) + mcl
                for half in range(2):
                    ps = psum_mm.tile([P, 512], F32, tag="mmps")
                    for p3 in range(PH):
                        nc.tensor.matmul(
                            ps[:, : D // 2],
                            lhsT=oT_sb[:, p3, mc * P:(mc + 1) * P],
                            rhs=wp_sb[:, p3, half * (D // 2):(half + 1) * (D // 2)],
                            start=(p3 == 0),
                            stop=(p3 == PH - 1),
                        )
                    oc = outcp.tile([P, D // 2], F32)
                    nc.vector.tensor_copy(oc[:, :], ps[:, : D // 2])
                    nc.sync.dma_start(
                        out=out[mc * P:(mc + 1) * P,
                                half * (D // 2):(half + 1) * (D // 2)],
                        in_=oc[:, :],
                    )

        def s_pair(nb, hp, mc):
            """S^T chunk mc for BOTH heads of plane hp: two K=64 row-tiled
            matmuls at partition bases 0 and 64 -- concurrent on the PE."""
            ps = psum_s.tile([P, 1024], F32, tag="sps")
            for j in range(2):
                kb = j * HD
                nc.tensor.matmul(
                    ps[:, j * 512:(j + 1) * 512],
                    lhsT=kT_sb[kb:kb + HD, hp, mc * P:(mc + 1) * P],
                    rhs=qT_sb[kb:kb + HD, hp, nb * 512:(nb + 1) * 512],
                    start=True,
                    stop=True,
                    tile_position=(kb, 0),
                )
            return ps

        # ---- minimal serial prologue, everything else drained JIT ----
        # kTz heads 0/1 for m<512 and Q^T plane 0 for n-block 0 are all the
        # first attention groups need; the rest of the K/V/Q production is
        # queued and emitted between attention groups so the PE produces
        # while ScalarE works through the exps.
        qk_proj(PH, 0)
        qk_proj(0, 0)

        extraq = []

        def drain(k):
            for _ in range(k):
                if extraq:
                    extraq.pop(0)()

        for mc in range(3):
            extraq.append(lambda mc=mc: v_proj(mc))
        extraq.append(lambda: qk_proj(PH, 1))
        for mc in range(3, 5):
            extraq.append(lambda mc=mc: v_proj(mc))
        extraq.append(lambda: qk_proj(0, 1))
        for mc in range(5, 7):
            extraq.append(lambda mc=mc: v_proj(mc))
        extraq.append(lambda: qk_proj(PH, 2))
        for mc in range(7, 10):
            extraq.append(lambda mc=mc: v_proj(mc))
        extraq.append(lambda: qk_proj(PH, 3))
        for mc in range(10, MC):
            extraq.append(lambda mc=mc: v_proj(mc))
        extraq.append(lambda: qk_proj(0, 2))
        extraq.append(lambda: qk_proj(0, 3))

        def proj_unit(nb, mcl, half):
            mc = nb * (512 // P) + mcl
            ps = psum_mm.tile([P, 512], F32, tag="mmps")
            p3s = (0, 1, 2)
            for i, p3 in enumerate(p3s):
                nc.tensor.matmul(
                    ps[:, : D // 2],
                    lhsT=oT_sb[:, p3, mc * P:(mc + 1) * P],
                    rhs=wp_sb[:, p3, half * (D // 2):(half + 1) * (D // 2)],
                    start=(i == 0),
                    stop=(i == PH - 1),
                )
            oc = outcp.tile([P, D // 2], F32)
            nc.vector.tensor_copy(oc[:, :], ps[:, : D // 2])
            nc.sync.dma_start(
                out=out[mc * P:(mc + 1) * P,
                        half * (D // 2):(half + 1) * (D // 2)],
                in_=oc[:, :],
            )

        # ---- attention: software-pipelined (nb, head-plane) blocks;
        # each block handles BOTH heads of one Q/K plane ----
        blocks = [(nb, hp) for hp in range(PH) for nb in range(NB)]
        pending = None
        for bi, (nb, hp) in enumerate(blocks):
            pT = ptp.tile([P, 2 * MC, 512], BF16)   # slot 2mc = head A, 2mc+1 = B
            poA = psum_o.tile([P, 512], F32, tag="po")
            poB = psum_o.tile([P, 512], F32, tag="po")
            hA, hB = 2 * hp, 2 * hp + 1
            def o_pair(mc, first):
                for po, h, slot in ((poA, hA, 2 * mc), (poB, hB, 2 * mc + 1)):
                    nc.tensor.matmul(
                        po[:, :],
                        lhsT=v_sb[:, mc, h * P:(h + 1) * P],
                        rhs=pT[:, slot, :],
                        start=first,
                        stop=(mc == MC - 1),
                    )

            # quads: two S-pairs back-to-back so the second pair's weight
            # loads hide under the first pair's streams
            for mc2 in range(0, MC, 2):
                if mc2 == 0 and pending is not None:
                    ps0 = pending
                    pending = None
                else:
                    ps0 = s_pair(nb, hp, mc2)
                ps1 = s_pair(nb, hp, mc2 + 1)
                for mc, ps in ((mc2, ps0), (mc2 + 1, ps1)):
                    nc.scalar.activation(
                        pT[:, 2 * mc:2 * mc + 2, :].rearrange("p a b -> p (a b)"),
                        ps[:, :],
                        mybir.ActivationFunctionType.Exp,
                        scale=SCALE,
                    )
                drain(3 if bi == 0 else 1)
                if mc2 >= 2:
                    o_pair(mc2 - 2, first=(mc2 == 2))
                    o_pair(mc2 - 1, first=False)
                if mc2 == MC - 2 and bi + 1 < len(blocks):
                    # next block's first S-pair ahead of this block's tail,
                    # so ScalarE never starves at the block boundary
                    nb2, hp2 = blocks[bi + 1]
                    pending = s_pair(nb2, hp2, 0)
            o_pair(MC - 2, first=False)
            o_pair(MC - 1, first=False)
            # normalize both heads: evacuate [65,512] out of PSUM promptly
            # (row 64 is the denominator, courtesy of the V ones-columns),
            # spread the 512 denominators over 64 partitions with a direct
            # SBUF->SBUF DMA, reciprocal there (DVE recip is ~8 cyc/elem, so
            # keep it to 8 elems/lane), and broadcast back via DRAM.  The two
            # heads' ops are interleaved so neither head's final multiply
            # blocks the other chain on the DVE FIFO.
            st = []
            for po, h in ((poA, hA), (poB, hB)):
                oTu = small.tile([HD + 1, 512], F32)
                nc.vector.tensor_copy(oTu[:, :], po[:HD + 1, :])
                st.append((h, oTu))
            for h, oTu in st:
                spread = small.tile([HD, 8], F32)
                nc.sync.dma_start(out=spread[:, :], in_=oTu[HD:HD + 1, :])
                st[(h % 2)] = (h, oTu, spread)
            for h, oTu, spread in st:
                rspread = small.tile([HD, 8], F32)
                nc.vector.reciprocal(rspread[:, :], spread[:, :])
                st[(h % 2)] = (h, oTu, rspread)
            for h, oTu, rspread in st:
                drcp = dramp.tile([1, 512], F32, tag="drcp")
                nc.sync.dma_start(
                    out=bass.AP(tensor=drcp.tensor, offset=drcp.offset,
                                ap=[[8, HD], [1, 8]]),
                    in_=rspread[:, :],
                )
                rcp = small.tile([HD, 512], F32)
                nc.sync.dma_start(
                    out=rcp[:, :],
                    in_=bass.AP(tensor=drcp.tensor, offset=drcp.offset,
                                ap=[[0, HD], [1, 512]]),
                )
                st[(h % 2)] = (h, oTu, rcp)
            for h, oTu, rcp in st:
                kb = (h % 2) * HD
                nc.vector.tensor_mul(
                    oT_sb[kb:kb + HD, hp, nb * 512:(nb + 1) * 512],
                    oTu[0:HD, :],
                    rcp[:, :],
                )
            # queue follow-on PE work: Q planes for the next n-block, and the
            # output projection for a completed n-block
            if hp + 1 < PH:
                extraq.append(lambda cb=PH + hp + 1, nbn=nb: qk_proj(cb, nbn))
                extraq.append(lambda cb=hp + 1, nbn=nb: qk_proj(cb, nbn))
            else:
                for mcl in range(512 // P):
                    for half in range(2):
                        extraq.append(
                            lambda nbp=nb, mcl=mcl, half=half: proj_unit(nbp, mcl, half)
                        )
        while extraq:
            extraq.pop(0)()

    _split_waits(nc)
    return nc


_NC_CACHE = [None]


def _get_nc():
    if _NC_CACHE[0] is None:
        _NC_CACHE[0] = _build_nc()
    return _NC_CACHE[0]


def _make_in_maps(x, W_qkv, W_proj):
    import ml_dtypes

    bf16 = ml_dtypes.bfloat16
    in_maps = []
    for c in range(NCORES):
        b = c // 2
        h0 = (c % 2) * HPC
        qcols = W_qkv[:, h0 * HD:(h0 + HPC) * HD]
        kcols = W_qkv[:, D + h0 * HD: D + (h0 + HPC) * HD]
        vcols = W_qkv[:, 2 * D + h0 * HD: 2 * D + (h0 + HPC) * HD]
        in_maps.append(
            {
                "xT": np.ascontiguousarray(x[b].T).astype(bf16),
                "wqkv": np.concatenate([qcols, kcols, vcols], axis=1).astype(bf16),
                "wproj": np.ascontiguousarray(
                    W_proj[h0 * HD:(h0 + HPC) * HD, :]
                ).astype(bf16),
            }
        )
    return in_maps


def _run(inputs, trace=False):
    x = np.asarray(inputs["x"], dtype=np.float32)
    W_qkv = np.asarray(inputs["W_qkv"], dtype=np.float32)
    W_proj = np.asarray(inputs["W_proj"], dtype=np.float32)
    b_proj = np.asarray(inputs["b_proj"], dtype=np.float32)

    if trace:
        _install_ntff_shim()
    nc = _get_nc()
    res = run_bass_kernel_spmd(
        nc, _make_in_maps(x, W_qkv, W_proj), core_ids=list(range(NCORES)),
        trace=trace,
    )
    parts = res.results
    out = np.empty((B, N, D), dtype=np.float32)
    for b in range(B):
        out[b] = parts[2 * b]["out"] + parts[2 * b + 1]["out"] + b_proj
    return out, res


def kernel(**inputs) -> np.ndarray:
    out, _ = _run(inputs, trace=False)
    return out


def run_traced(inputs):
    return _run(inputs, trace=True)



# revision 33
# speedup vs baseline: 1.3939x; 1.3939x over previous
"""Trainium2 Bass kernel for nn_Attention_42417097015520.

Full-input contract: kernel(**inputs) takes the unsharded inputs
(x [4,2048,768], W_qkv [768,2304], W_proj [768,768], b_proj [768]) and
returns the full [4,2048,768] output.

Sharding (8 cores): core c handles batch b=c//2 and heads
h in [(c%2)*6, (c%2)*6+6) (tensor parallel over heads x data parallel
over batch). Each core computes its 6 heads' attention plus the partial
output projection against its 384-row slice of W_proj; the host sums the
two partials per batch and adds b_proj.

Device-side layout/algorithm (per core, identical SPMD program, bf16
matmul operands, fp32 PSUM accumulation):
  - inputs: xT = x[b].T [768,2048], wqkv = W_qkv column slice [768,1152]
    (q|k|v blocks of 384), wproj row slice [384,768].
  - Q^T/K^T projections in head-pair-packed planes (head h on partitions
    (h%2)*64.. of plane h//2); V with a ones column per head (so the P@V
    matmul also emits the softmax denominators as PSUM row 64) and junk
    padding so its lhsT can be taken 128 wide (full PE array keeps the
    HAM clock-gate warm).
  - Attention blocks (n-block x head-plane): S^T chunks for both heads of
    a plane issued as row-tiled matmuls at partition bases 0/64 (disjoint
    row groups run concurrently), exp on ScalarE (scale=1/8 folded in,
    two chunks per activation), O^T accumulated over m-chunks, then
    normalized by the reciprocal denominator row broadcast via a DRAM
    bounce. All remaining production (K/V/Q projections, output
    projection) is drained as "extra PE work" between attention groups
    from a JIT queue, and each block's first S-matmuls are emitted ahead
    of the previous block's tail, so ScalarE (the ~200us exp floor) and
    the PE stay busy simultaneously.
  - Output projection from the O^T layout (heads on partitions), partial
    result [2048,768] DMA'd out; host sums batch partials + b_proj.
"""

import sys
import types
import contextlib
import ctypes
from contextlib import ExitStack

import numpy as np

import concourse.bass as bass
import concourse.mybir as mybir
import concourse.tile as tile
from concourse.bass_utils import run_bass_kernel_spmd

B, N, D, H, HD = 4, 2048, 768, 12, 64
HPC = H // 2          # heads per core = 6
NCORES = 8
SCALE = HD ** -0.5    # 0.125
F32 = mybir.dt.float32
F32R = mybir.dt.float32r
BF16 = mybir.dt.bfloat16
P = 128
VW = P                # V columns per head: 64 data + 64 ones (denominator rows)


# ---------------------------------------------------------------------------
# Workaround: this container's walrus accepts at most ONE sem wait per
# instruction. Hoist extra waits onto same-engine NoOps inserted before.
# ---------------------------------------------------------------------------
_wsplit_ctr = [0]


def _split_waits(nc, cap: int = 1) -> int:
    n_split = 0
    for f in nc.m.functions:
        for bb in f.blocks:
            insts = list(bb.instructions)
            out = []
            for ins in insts:
                si = ins.sync_info
                if si is not None and si.on_wait and len(si.on_wait) > cap:
                    waits = list(si.on_wait)
                    for i in range(0, len(waits) - cap, cap):
                        _wsplit_ctr[0] += 1
                        out.append(
                            mybir.InstNoOp(
                                name=f"I-wsplit-{_wsplit_ctr[0]}",
                                engine=ins.engine,
                                ins=[],
                                outs=[],
                                sync_info=mybir.SyncInfo(
                                    on_wait=waits[i : i + cap], on_update=[]
                                ),
                            )
                        )
                    si.on_wait = waits[len(waits) - cap :]
                    n_split += 1
                out.append(ins)
            if len(out) != len(insts):
                bb.instructions[:] = out
    return n_split


# ---------------------------------------------------------------------------
# Post-pass: every matmul/activation/copy carries a +1 sem update, but only
# the counts some wait actually references need an update (each serialized
# inc costs ~26 ns on the engine).  Keep updates only at wait thresholds,
# bumping their increment to cover the dropped ones, so every wait fires at
# exactly the same instruction completion as before.
# ---------------------------------------------------------------------------
def _coalesce_sem_updates(nc, sems=("PE_44", "Activation_44", "DVE_44")):
    thr = {s: set() for s in sems}
    ok = {s: True for s in sems}
    streams = {s: [] for s in sems}
    waiters = {s: [] for s in sems}
    engines = {s: set() for s in sems}
    for f in nc.m.functions:
        for bb in f.blocks:
            for ins in bb.instructions:
                si = ins.sync_info
                if not si:
                    continue
                for w in (si.on_wait or []):
                    if w.ant_name in thr:
                        if w.wait_mode != "sem-ge-imm" or w.wait_reg is not None:
                            ok[w.ant_name] = False
                        else:
                            thr[w.ant_name].add(w.wait_value)
                            waiters[w.ant_name].append(w)
                for u in (si.on_update or []):
                    if u.ant_name in streams:
                        if u.update_mode != "sem-inc" or u.update_value != 1:
                            ok[u.ant_name] = False
                        streams[u.ant_name].append((ins, u))
                        engines[u.ant_name].add(str(ins.engine))
    removed = 0
    # Keep a +1 update only at counts some wait references, and renumber all
    # wait values to the rank of their threshold.  Updates stay attached to
    # their original instruction (so they still fire at its completion) and
    # stay sem-inc (no add-imm, which shares the ISA immediate with ge-imm
    # waits); every wait fires at exactly the same completion as before.
    for s in sems:
        lst = streams[s]
        if not ok[s] or len(engines[s]) != 1 or not lst:
            continue
        total = len(lst)
        T = sorted(v for v in thr[s] if 0 < v <= total)
        rank = {v: i + 1 for i, v in enumerate(T)}
        keep = set(T)
        for c, (ins, u) in enumerate(lst, start=1):
            if c not in keep:
                ins.sync_info.on_update = [
                    x for x in ins.sync_info.on_update if x is not u
                ]
                removed += 1
        seen = set()
        for w in waiters[s]:
            if id(w) in seen:
                continue
            seen.add(id(w))
            if 0 < w.wait_value <= total:
                w.wait_value = rank[w.wait_value]
    return removed


# ---------------------------------------------------------------------------
# Post-pass: each S-pair emits two half-height (64,128) LDWEIGHTS for the
# row-tiled matmuls; a half-height load is not FWL-eligible and cannot hide
# behind a full-height matmul (row conflict), exposing ~107ns per pair.
# Merge each pair into ONE full-height (128,128) LDWEIGHTS covering both
# tiles' rows (the same contiguous SBUF region), which hides like the O
# matmuls' loads do.
# ---------------------------------------------------------------------------
def _merge_s_ldw(nc):
    n = 0
    for f in nc.m.functions:
        for bb in f.blocks:
            insts = bb.instructions
            out = []
            i = 0
            while i < len(insts):
                ins = insts[i]
                if (
                    type(ins).__name__ == "InstLdweights"
                    and getattr(ins, "tile_size", None) == (64, 128)
                    and ins.tile_position == (0, 0)
                ):
                    stride = ins.ins[0].ap[0][0]
                    for j in range(i + 1, min(i + 5, len(insts))):
                        p = insts[j]
                        if (
                            type(p).__name__ == "InstLdweights"
                            and getattr(p, "tile_size", None) == (64, 128)
                            and p.tile_position == (64, 0)
                            and p.ins[0].offset == ins.ins[0].offset + 64 * stride
                            and (p.sync_info is None
                                 or (not p.sync_info.on_wait
                                     and not p.sync_info.on_update))
                        ):
                            break
                    else:
                        out.append(ins)
                        i += 1
                        continue
                    ins.ins[0].ap = [[stride, 128], [1, 128]]
                    ins.tile_size = (128, 128)
                    out.append(ins)
                    for k in range(i + 1, j):
                        out.append(insts[k])
                    i = j + 1
                    n += 1
                    continue
                out.append(ins)
                i += 1
            bb.instructions[:] = out
    return n


# ---------------------------------------------------------------------------
# NTFF profiling shim (the image's antenv lacks axon_hooks); only needed
# when trace=True is requested.
# ---------------------------------------------------------------------------
_HOOK = [None]


def _install_ntff_shim():
    if "antenv.axon_hooks" in sys.modules:
        return
    mod = types.ModuleType("antenv.axon_hooks")
    mod.set_axon_ntff_profile_hook = lambda h: _HOOK.__setitem__(0, h)
    mod.get_axon_ntff_profile_hook = lambda: _HOOK[0]
    sys.modules["antenv.axon_hooks"] = mod
    try:
        import antenv

        antenv.axon_hooks = mod
    except ImportError:
        pass

    try:
        lib = ctypes.CDLL("/opt/axon/libaxon_pjrt.so")
    except OSError:
        return
    if not hasattr(lib, "axon_start_nrt_profile"):
        return
    lib.axon_start_nrt_profile.argtypes = [
        ctypes.POINTER(ctypes.c_int64),
        ctypes.c_size_t,
    ]
    lib.axon_start_nrt_profile.restype = ctypes.c_int64
    lib.axon_stop_nrt_profile.argtypes = [ctypes.c_char_p]
    lib.axon_stop_nrt_profile.restype = ctypes.c_int64

    @contextlib.contextmanager
    def _hook(output_dir, device_ids):
        import jax

        jax.devices()
        if device_ids:
            ids = (ctypes.c_int64 * len(device_ids))(*device_ids)
            rc = lib.axon_start_nrt_profile(ids, len(device_ids))
        else:
            rc = lib.axon_start_nrt_profile(None, 0)
        if rc != 0:
            raise RuntimeError(f"axon_start_nrt_profile rc={rc}")
        try:
            yield
        finally:
            n = lib.axon_stop_nrt_profile(str(output_dir).encode())
            if n < 0:
                raise RuntimeError(f"axon_stop_nrt_profile rc={n}")

    _HOOK[0] = _hook

    import concourse.bass_utils as bu

    bu.upload_artifacts = lambda tmpdir: str(tmpdir)


# ---------------------------------------------------------------------------
# Device program
# ---------------------------------------------------------------------------
def _build_nc():
    nc = bass.Bass()
    xT = nc.declare_dram_parameter("xT", [D, N], BF16, isOutput=False).ap()
    wqkv = nc.declare_dram_parameter("wqkv", [D, 3 * HPC * HD], BF16, isOutput=False).ap()
    wproj = nc.declare_dram_parameter("wproj", [HPC * HD, D], BF16, isOutput=False).ap()
    out = nc.declare_dram_parameter("out", [N, D], F32, isOutput=True).ap()

    DO = D // P          # 6 d-chunks of 128
    NB = N // 512        # 4 n-blocks of 512
    MC = N // P          # 16 m-chunks of 128
    PH = HPC * HD // P   # 3 planes of head-dims

    with tile.TileContext(nc) as tc, ExitStack() as ctx:
        persist = ctx.enter_context(tc.tile_pool(name="persist", bufs=1))
        ptp = ctx.enter_context(tc.tile_pool(name="ptp", bufs=2))
        outcp = ctx.enter_context(tc.tile_pool(name="outcp", bufs=3))
        small = ctx.enter_context(tc.tile_pool(name="small", bufs=8))
        dramp = ctx.enter_context(tc.tile_pool(name="dramp", bufs=8, space="DRAM"))
        psum_mm = ctx.enter_context(tc.tile_pool(name="psum_mm", bufs=2, space="PSUM"))
        psum_s = ctx.enter_context(tc.tile_pool(name="psum_s", bufs=2, space="PSUM"))
        psum_o = ctx.enter_context(tc.tile_pool(name="psum_o", bufs=2, space="PSUM"))

        # Q^T and K^T planes use head-pair packing: head h lives on
        # partitions (h%2)*64.. of plane h//2. The S^T matmuls for the two
        # heads of a plane are emitted back-to-back as row-tiled (base
        # partition 0 / 64) matmuls, so they run CONCURRENTLY on disjoint
        # row groups of the PE array -- 2x throughput, and the combined
        # activity keeps the HAM clock-gate at full speed.
        qT_sb = persist.tile([P, PH, N], BF16)                   # [128, 3, 2048]
        kT_sb = persist.tile([P, PH, N], BF16)                   # [128, 3, 2048]
        # V tile: per head 64 V columns + 64 ONES columns, so each O^T matmul
        # (lhsT 128 wide) emits the softmax denominator replicated on PSUM
        # rows 64..127 for free -- normalization then needs no cross-partition
        # broadcast, just a DVE reciprocal + multiply straight from PSUM.
        v_sb = persist.tile([P, MC, HPC * P], BF16)              # [128, 16, 768]
        oT_sb = persist.tile([P, PH, N], BF16)                   # [128, 3, 2048]
        wp_sb = persist.tile([P, PH, D], BF16)                   # [128, 3, 768]
        xT_sb = persist.tile([P, DO, N], BF16)                   # [128, 6, 2048]
        wqkv_sb = persist.tile([P, DO, 3 * HPC * HD], BF16)      # [128, 6, 1152]

        # hoist the ACT table load (~2.7us) into the DMA prologue; its memset
        # must be DVE's first op so the dummy activation issues immediately
        warm = small.tile([1, 8], F32)
        nc.vector.memset(warm[:, :], 0.0)
        nc.scalar.activation(warm[:, :], warm[:, :],
                             mybir.ActivationFunctionType.Exp, scale=1.0)

        for h in range(HPC):
            nc.vector.memset(v_sb[:, :, h * P + HD:(h + 1) * P], 1.0)

        # DMA order: first the K-plane-0 wqkv slice (cols 384:512, the first
        # matmul's weights) and the first x quarter, then Q cols, then the
        # remaining K planes, V, and the rest of x.
        QK = 2 * HPC * HD
        for o in range(DO):
            nc.gpsimd.dma_start(out=wqkv_sb[:, o, PH * P:(PH + 1) * P],
                                in_=wqkv[o * P:(o + 1) * P, PH * P:(PH + 1) * P])
            nc.sync.dma_start(out=xT_sb[:, o, 0:512], in_=xT[o * P:(o + 1) * P, 0:512])
        for o in range(DO):
            nc.gpsimd.dma_start(out=wqkv_sb[:, o, 0:PH * P],
                                in_=wqkv[o * P:(o + 1) * P, 0:PH * P])
        for o in range(DO):
            nc.gpsimd.dma_start(out=wqkv_sb[:, o, (PH + 1) * P:QK],
                                in_=wqkv[o * P:(o + 1) * P, (PH + 1) * P:QK])
        for o in range(DO):
            nc.gpsimd.dma_start(out=wqkv_sb[:, o, QK:], in_=wqkv[o * P:(o + 1) * P, QK:])
        for o in range(DO):
            nc.sync.dma_start(out=xT_sb[:, o, 512:N // 2], in_=xT[o * P:(o + 1) * P, 512:N // 2])
        for o in range(DO):
            nc.sync.dma_start(out=xT_sb[:, o, N // 2:N], in_=xT[o * P:(o + 1) * P, N // 2:N])
        for p3 in range(PH):
            nc.sync.dma_start(out=wp_sb[:, p3, :], in_=wproj[p3 * P:(p3 + 1) * P, :])

        def qk_proj(cb, nb):
            """Produce one [128,512] column-block of Q^T (cb<3) or K^T."""
            ps = psum_mm.tile([P, 512], F32, tag="mmps")
            for o in range(DO):
                nc.tensor.matmul(
                    ps[:, :],
                    lhsT=wqkv_sb[:, o, cb * P:(cb + 1) * P],
                    rhs=xT_sb[:, o, nb * 512:(nb + 1) * 512],
                    start=(o == 0),
                    stop=(o == DO - 1),
                )
            sl = slice(nb * 512, (nb + 1) * 512)
            if cb < PH:
                nc.vector.tensor_copy(qT_sb[:, cb, sl], ps[:, :])
            else:
                nc.vector.tensor_copy(kT_sb[:, cb - PH, sl], ps[:, :])

        def v_proj(mc):
            ps = psum_mm.tile([P, 512], F32, tag="mmps")
            for o in range(DO):
                nc.tensor.matmul(
                    ps[:, : HPC * HD],
                    lhsT=xT_sb[:, o, mc * P:(mc + 1) * P],
                    rhs=wqkv_sb[:, o, 2 * HPC * HD: 3 * HPC * HD],
                    start=(o == 0),
                    stop=(o == DO - 1),
                )
            nc.vector.tensor_copy(
                v_sb.rearrange("p m (h c) -> p m h c", c=P)[:, mc, :, 0:HD],
                ps[:, : HPC * HD].rearrange("p (h c) -> p h c", c=HD),
            )

        def proj(nb):
            """Output projection for one 512-row n-block."""
            for mcl in range(512 // P):
                mc = nb * (512 // P) + mcl
                for half in range(2):
                    ps = psum_mm.tile([P, 512], F32, tag="mmps")
                    for p3 in range(PH):
                        nc.tensor.matmul(
                            ps[:, : D // 2],
                            lhsT=oT_sb[:, p3, mc * P:(mc + 1) * P],
                            rhs=wp_sb[:, p3, half * (D // 2):(half + 1) * (D // 2)],
                            start=(p3 == 0),
                            stop=(p3 == PH - 1),
                        )
                    oc = outcp.tile([P, D // 2], F32)
                    nc.vector.tensor_copy(oc[:, :], ps[:, : D // 2])
                    nc.sync.dma_start(
                        out=out[mc * P:(mc + 1) * P,
                                half * (D // 2):(half + 1) * (D // 2)],
                        in_=oc[:, :],
                    )

        def s_pair(nb, hp, mc):
            """S^T chunk mc for BOTH heads of plane hp: two K=64 row-tiled
            matmuls at partition bases 0 and 64 -- concurrent on the PE."""
            ps = psum_s.tile([P, 1024], F32, tag="sps")
            for j in range(2):
                kb = j * HD
                nc.tensor.matmul(
                    ps[:, j * 512:(j + 1) * 512],
                    lhsT=kT_sb[kb:kb + HD, hp, mc * P:(mc + 1) * P],
                    rhs=qT_sb[kb:kb + HD, hp, nb * 512:(nb + 1) * 512],
                    start=True,
                    stop=True,
                    tile_position=(kb, 0),
                )
            return ps

        # ---- minimal serial prologue, everything else drained JIT ----
        # kTz heads 0/1 for m<512 and Q^T plane 0 for n-block 0 are all the
        # first attention groups need; the rest of the K/V/Q production is
        # queued and emitted between attention groups so the PE produces
        # while ScalarE works through the exps.
        qk_proj(PH, 0)
        qk_proj(0, 0)

        extraq = []

        def drain(k):
            for _ in range(k):
                if extraq:
                    extraq.pop(0)()

        for mc in range(3):
            extraq.append(lambda mc=mc: v_proj(mc))
        extraq.append(lambda: qk_proj(PH, 1))
        for mc in range(3, 5):
            extraq.append(lambda mc=mc: v_proj(mc))
        extraq.append(lambda: qk_proj(0, 1))
        for mc in range(5, 7):
            extraq.append(lambda mc=mc: v_proj(mc))
        extraq.append(lambda: qk_proj(PH, 2))
        for mc in range(7, 10):
            extraq.append(lambda mc=mc: v_proj(mc))
        extraq.append(lambda: qk_proj(PH, 3))
        for mc in range(10, MC):
            extraq.append(lambda mc=mc: v_proj(mc))
        extraq.append(lambda: qk_proj(0, 2))
        extraq.append(lambda: qk_proj(0, 3))

        def proj_unit(nb, mcl, half):
            mc = nb * (512 // P) + mcl
            ps = psum_mm.tile([P, 512], F32, tag="mmps")
            p3s = (0, 1, 2)
            for i, p3 in enumerate(p3s):
                nc.tensor.matmul(
                    ps[:, : D // 2],
                    lhsT=oT_sb[:, p3, mc * P:(mc + 1) * P],
                    rhs=wp_sb[:, p3, half * (D // 2):(half + 1) * (D // 2)],
                    start=(i == 0),
                    stop=(i == PH - 1),
                )
            oc = outcp.tile([P, D // 2], F32)
            nc.vector.tensor_copy(oc[:, :], ps[:, : D // 2])
            nc.sync.dma_start(
                out=out[mc * P:(mc + 1) * P,
                        half * (D // 2):(half + 1) * (D // 2)],
                in_=oc[:, :],
            )

        # ---- attention: software-pipelined (nb, head-plane) blocks;
        # each block handles BOTH heads of one Q/K plane ----
        blocks = [(nb, hp) for hp in range(PH) for nb in range(NB)]
        pending = None
        for bi, (nb, hp) in enumerate(blocks):
            pT = ptp.tile([P, 2 * MC, 512], BF16)   # slot 2mc = head A, 2mc+1 = B
            poA = psum_o.tile([P, 512], F32, tag="po")
            poB = psum_o.tile([P, 512], F32, tag="po")
            hA, hB = 2 * hp, 2 * hp + 1
            def o_pair(mc, first):
                for po, h, slot in ((poA, hA, 2 * mc), (poB, hB, 2 * mc + 1)):
                    nc.tensor.matmul(
                        po[:, :],
                        lhsT=v_sb[:, mc, h * P:(h + 1) * P],
                        rhs=pT[:, slot, :],
                        start=first,
                        stop=(mc == MC - 1),
                    )

            # quads: two S-pairs back-to-back so the second pair's weight
            # loads hide under the first pair's streams
            for mc2 in range(0, MC, 2):
                if mc2 == 0 and pending is not None:
                    ps0 = pending
                    pending = None
                else:
                    ps0 = s_pair(nb, hp, mc2)
                ps1 = s_pair(nb, hp, mc2 + 1)
                for mc, ps in ((mc2, ps0), (mc2 + 1, ps1)):
                    nc.scalar.activation(
                        pT[:, 2 * mc:2 * mc + 2, :].rearrange("p a b -> p (a b)"),
                        ps[:, :],
                        mybir.ActivationFunctionType.Exp,
                        scale=SCALE,
                    )
                drain(3 if bi == 0 else 1)
                if mc2 >= 2:
                    o_pair(mc2 - 2, first=(mc2 == 2))
                    o_pair(mc2 - 1, first=False)
                if mc2 == MC - 2 and bi + 1 < len(blocks):
                    # next block's first S-pair ahead of this block's tail,
                    # so ScalarE never starves at the block boundary
                    nb2, hp2 = blocks[bi + 1]
                    pending = s_pair(nb2, hp2, 0)
            o_pair(MC - 2, first=False)
            o_pair(MC - 1, first=False)
            # normalize both heads: evacuate [65,512] out of PSUM promptly
            # (row 64 is the denominator, courtesy of the V ones-columns),
            # spread the 512 denominators over 64 partitions with a direct
            # SBUF->SBUF DMA, reciprocal there (DVE recip is ~8 cyc/elem, so
            # keep it to 8 elems/lane), and broadcast back via DRAM.  The two
            # heads' ops are interleaved so neither head's final multiply
            # blocks the other chain on the DVE FIFO.
            st = []
            for po, h in ((poA, hA), (poB, hB)):
                oTu = small.tile([HD + 1, 512], F32)
                nc.vector.tensor_copy(oTu[:, :], po[:HD + 1, :])
                st.append((h, oTu))
            for h, oTu in st:
                spread = small.tile([HD, 8], F32)
                nc.sync.dma_start(out=spread[:, :], in_=oTu[HD:HD + 1, :])
                st[(h % 2)] = (h, oTu, spread)
            for h, oTu, spread in st:
                rspread = small.tile([HD, 8], F32)
                nc.vector.reciprocal(rspread[:, :], spread[:, :])
                st[(h % 2)] = (h, oTu, rspread)
            for h, oTu, rspread in st:
                drcp = dramp.tile([1, 512], F32, tag="drcp")
                nc.sync.dma_start(
                    out=bass.AP(tensor=drcp.tensor, offset=drcp.offset,
                                ap=[[8, HD], [1, 8]]),
                    in_=rspread[:, :],
                )
                rcp = small.tile([HD, 512], F32)
                nc.sync.dma_start(
                    out=rcp[:, :],
                    in_=bass.AP(tensor=drcp.tensor, offset=drcp.offset,
                                ap=[[0, HD], [1, 512]]),
                )
                st[(h % 2)] = (h, oTu, rcp)
            for h, oTu, rcp in st:
                kb = (h % 2) * HD
                nc.vector.tensor_mul(
                    oT_sb[kb:kb + HD, hp, nb * 512:(nb + 1) * 512],
                    oTu[0:HD, :],
                    rcp[:, :],
                )
            # queue follow-on PE work: Q planes for the next n-block, and the
            # output projection for a completed n-block
            if hp + 1 < PH:
                extraq.append(lambda cb=PH + hp + 1, nbn=nb: qk_proj(cb, nbn))
                extraq.append(lambda cb=hp + 1, nbn=nb: qk_proj(cb, nbn))
            else:
                for mcl in range(512 // P):
                    for half in range(2):
                        extraq.append(
                            lambda nbp=nb, mcl=mcl, half=half: proj_unit(nbp, mcl, half)
                        )
        while extraq:
            extraq.pop(0)()

    _split_waits(nc)
    return nc


_NC_CACHE = [None]


def _get_nc():
    if _NC_CACHE[0] is None:
        _NC_CACHE[0] = _build_nc()
    return _NC_CACHE[0]


def _make_in_maps(x, W_qkv, W_proj):
    import ml_dtypes

    bf16 = ml_dtypes.bfloat16
    in_maps = []
    for c in range(NCORES):
        b = c // 2
        h0 = (c % 2) * HPC
        qcols = W_qkv[:, h0 * HD:(h0 + HPC) * HD]
        kcols = W_qkv[:, D + h0 * HD: D + (h0 + HPC) * HD]
        vcols = W_qkv[:, 2 * D + h0 * HD: 2 * D + (h0 + HPC) * HD]
        in_maps.append(
            {
                "xT": np.ascontiguousarray(x[b].T).astype(bf16),
                "wqkv": np.concatenate([qcols, kcols, vcols], axis=1).astype(bf16),
                "wproj": np.ascontiguousarray(
                    W_proj[h0 * HD:(h0 + HPC) * HD, :]
                ).astype(bf16),
            }
        )
    return in_maps


def _run(inputs, trace=False):
    x = np.asarray(inputs["x"], dtype=np.float32)
    W_qkv = np.asarray(inputs["W_qkv"], dtype=np.float32)
    W_proj = np.asarray(inputs["W_proj"], dtype=np.float32)
    b_proj = np.asarray(inputs["b_proj"], dtype=np.float32)

    if trace:
        _install_ntff_shim()
    nc = _get_nc()
    res = run_bass_kernel_spmd(
        nc, _make_in_maps(x, W_qkv, W_proj), core_ids=list(range(NCORES)),
        trace=trace,
    )
    parts = res.results
    out = np.empty((B, N, D), dtype=np.float32)
    for b in range(B):
        out[b] = parts[2 * b]["out"] + parts[2 * b + 1]["out"] + b_proj
    return out, res


def kernel(**inputs) -> np.ndarray:
    out, _ = _run(inputs, trace=False)
    return out


def run_traced(inputs):
    return _run(inputs, trace=True)

